# revision 21
# baseline (speedup 1.0000x reference)
"""Single-head causal attention on 8 TRN2 NeuronCores (Bass/Tile).

Problem: x[B=8,T=4096,C=1024] @ {Wq,Wk,Wv}[C,HS=64] -> causal softmax
attention -> out[B,T,HS].

Sharding: data-parallel over batch — core b computes batch element b with
replicated projection weights (per the sharding hint).

Schedule (v3 — DVFS/LDW-aware): the TRN2 PE only reaches its 2.4 GHz
p-state after ~3us of continuous execution, and without walrus ldw-opt
(whose LDWEIGHTS double-buffering is incompatible with Tile's semaphores)
every matmul pays a serialized K-row weight load. The kernel is one dense
software-pipelined PE stream matched against the ScalarE exp wall (~75us),
with weight-load cost minimized structurally:

  - xT arrives bf16 (host-cast): halves HBM traffic, no on-chip casts.
  - Per 512-wide query block: [qT;kT] = [Wq|Wk]^T @ xT and vT = Wv^T @ xT
    (both N=512 streams, 128-row weight loads). vT is restored to natural
    v[s,64] by a DMA XBAR transpose straight into v_all (no PE transpose);
    a ones column rides v_all so PV emits softmax row sums for free.
  - kT is interleaved into pair layout (tile 2p on partitions 0:64, 2p+1
    on 64:128) and q duplicated onto the upper half, so each full pair of
    key tiles runs as two CONCURRENT K=64 matmuls on disjoint PE row
    groups (~2x QK throughput). Diagonal pairs run unpacked with shifted
    PSUM columns so the valid region stays contiguous: exactly one
    ScalarE exp instruction per pair (scale folded in; no running max
    needed — logits are small by construction).
  - Causal masking: only the 128x128 diagonal strips, via gpsimd multiply
    with an upper-triangular tile (SBUF-only; gpsimd cannot touch PSUM).
  - PV accumulates outT[65,512] over s-tiles. Finalize is PE-free: DVE
    copies outT to bf16, a DMA XBAR transpose restores [t,h] order, DVE
    normalizes by the transposed row sums, stores write 1KB/partition.
  - Pair pipeline: QK runs LAG=2 pairs ahead of PV (ps_wei bufs=2), and
    next-block projection matmuls fill PE slack between pairs, so the PE
    almost never waits on ScalarE and the DVFS ramp holds.
  - Queues: all streaming DMA rides the sync (SP) queue in data-ready
    order; only the tiny constant loads share the ScalarE queue, keeping
    the exp stream unpolluted.
"""

import numpy as np

import concourse.bacc as bacc
import concourse.bass as bass
import concourse.mybir as mybir
import concourse.tile as tile
from concourse import bass_utils

B, T, C, HS = 8, 4096, 1024, 64
TB = 512                 # query-block width (PSUM bank = 512 fp32)
NJ = T // TB             # 8 query blocks
NK = C // 128            # 8 contraction chunks
NS = T // 128            # 32 key tiles
SCALE = C ** -0.5
LAG = 2                  # pairs QK runs ahead of PV

F32 = mybir.dt.float32
BF16 = mybir.dt.bfloat16
EXP = mybir.ActivationFunctionType.Exp


def build_program():
    nc = bacc.Bacc("TRN2", target_bir_lowering=False, debug=False)

    xTp = nc.dram_tensor("xTp", [NJ, 128, NK * TB], BF16, kind="ExternalInput")
    wqk = nc.dram_tensor("wqk", [128, NK * 128], BF16, kind="ExternalInput")
    wv = nc.dram_tensor("wv", [128, NK * HS], BF16, kind="ExternalInput")
    mask = nc.dram_tensor("mask", [128, 128], BF16, kind="ExternalInput")
    out = nc.dram_tensor("out", [T, HS], F32, kind="ExternalOutput")

    with tile.TileContext(nc) as tc:
        with (
            tc.tile_pool(name="const", bufs=1) as constp,
            tc.tile_pool(name="xt", bufs=1) as xtp,
            tc.tile_pool(name="qkt", bufs=1) as qktp,
            tc.tile_pool(name="persist", bufs=1) as persist,
            tc.tile_pool(name="vts", bufs=2) as vtsp,
            tc.tile_pool(name="expp", bufs=6) as expp,
            tc.tile_pool(name="fin", bufs=2) as finp,
            tc.tile_pool(name="ps_wei", bufs=2, space=bass.MemorySpace.PSUM) as ps_wei,
            tc.tile_pool(name="ps_qk", bufs=1, space=bass.MemorySpace.PSUM) as ps_qk,
            tc.tile_pool(name="ps_vt", bufs=1, space=bass.MemorySpace.PSUM) as ps_vt,
            tc.tile_pool(name="ps_out", bufs=2, space=bass.MemorySpace.PSUM) as ps_out,
        ):
            wqk_sb = constp.tile([128, NK, 128], BF16)
            wv_sb = constp.tile([128, NK, HS], BF16)
            mask_sb = constp.tile([128, 128], BF16)
            nc.scalar.dma_start(
                wqk_sb[:], wqk[:].rearrange("p (k m) -> p k m", k=NK)
            )
            nc.scalar.dma_start(
                wv_sb[:], wv[:].rearrange("p (k m) -> p k m", k=NK)
            )
            nc.scalar.dma_start(mask_sb[:], mask[:])
            # tiny dummy exp: pulls ACT_TABLE_LOAD (~2.7us) into the DMA head
            warm = constp.tile([1, 1], F32)
            nc.scalar.activation(warm[:], mask_sb[0:1, 0:1], EXP, scale=SCALE)

            # x (bf16, transposed): one tile per block; first two loads up
            # front, the rest staggered per-iteration to keep the sync
            # queue in data-need order
            xts = [None] * NJ

            def load_x(j):
                xt = xtp.tile([128, NK, TB], BF16, tag=f"xt{j}", name=f"xt{j}")
                nc.sync.dma_start(
                    xt[:], xTp[j].rearrange("p (k t) -> p k t", k=NK)
                )
                xts[j] = xt

            load_x(0)
            load_x(1)

            # persistent per-block [qT;kT] (rows 0:64 q, 64:128 k)
            qkts = [
                qktp.tile([128, TB], BF16, tag=f"qkt{j}", name=f"qkt{j}")
                for j in range(NJ)
            ]
            # q duplicated onto partitions 64:128 (rhs for packed B-half)
            q2s = [
                qktp.tile([128, TB], BF16, tag=f"q2_{j}", name=f"q2_{j}")
                for j in range(NJ)
            ]
            # kT interleaved: pair p = s-tile 2p on partitions 0:64, s-tile
            # 2p+1 on partitions 64:128
            kTI = persist.tile([128, (NS // 2) * 128], BF16)
            # values in natural [s,h] + ones column for row sums
            v_all = persist.tile([128, NS, HS + 1], BF16)
            nc.vector.memset(v_all[:, :, HS : HS + 1], 1.0)
            # finalize staging (padded to 80 partitions for the XBAR)
            outT_sbs = [
                persist.tile([80, TB], BF16, tag=f"oT{i}", name=f"oT{i}")
                for i in range(2)
            ]
            nc.gpsimd.memset(outT_sbs[0][64:80, :], 0.0)
            nc.gpsimd.memset(outT_sbs[1][64:80, :], 0.0)

            def drain_qk(j, qk_tile):
                """q/k copies + reshuffle DMAs: fires right after block j's
                qk-proj matmuls so the next iteration's q operands arrive
                with half an iteration of slack."""
                nc.vector.tensor_copy(qkts[j][:], qk_tile[:])
                # kT interleave: even tiles -> partitions 0:64, odd -> 64:128
                kt_src = qkts[j][64:128, :].rearrange(
                    "p (a e b) -> p a e b", e=2, b=128
                )
                kt_dst = kTI[:, 256 * j : 256 * (j + 1)].rearrange(
                    "p (a b) -> p a b", b=128
                )
                nc.sync.dma_start(kt_dst[0:64, :, :], kt_src[:, :, 0, :])
                nc.sync.dma_start(kt_dst[64:128, :, :], kt_src[:, :, 1, :])
                # q duplicated to the upper half for the packed B matmul
                nc.sync.dma_start(q2s[j][64:128, :], qkts[j][0:64, :])

            deferred = []  # gpsimd copies issued at end-of-iter so they
            # never head-block the pair masks on the gpsimd queue

            def drain_v(j, vt_tile):
                """vT -> natural v tiles via XBAR ([64,512] -> [128,4,64]);
                contiguous XBAR dst, then a strided copy into v_all (the
                XBAR can't scatter around the ones column)."""
                vt_sb = vtsp.tile([HS, TB], BF16, tag="vt_sb")
                nc.vector.tensor_copy(vt_sb[:], vt_tile[:])
                vnat = vtsp.tile([128, 4, HS], BF16, tag="vnat")
                nc.sync.dma_start_transpose(vnat[:], vt_sb[:])
                deferred.append(
                    lambda j=j, vnat=vnat: nc.gpsimd.tensor_copy(
                        v_all[:, 4 * j : 4 * j + 4, 0:HS], vnat[:]
                    )
                )

            def proj_items(j):
                """PE filler for block j's projections, with the engine-side
                drains woven in right after their producing matmuls."""
                items = []
                qk_tile = ps_qk.tile([128, TB], F32, tag="qk")
                for k in range(NK):
                    items.append(
                        lambda k=k, qk_tile=qk_tile, j=j: nc.tensor.matmul(
                            qk_tile[:], wqk_sb[:, k, :], xts[j][:, k, :],
                            start=(k == 0), stop=(k == NK - 1),
                        )
                    )
                items.append(lambda j=j, qk_tile=qk_tile: drain_qk(j, qk_tile))
                vt_tile = ps_vt.tile([HS, TB], F32, tag="vt")
                for k in range(NK):
                    items.append(
                        lambda k=k, vt_tile=vt_tile, j=j: nc.tensor.matmul(
                            vt_tile[:], wv_sb[:, k, :], xts[j][:, k, :],
                            start=(k == 0), stop=(k == NK - 1),
                        )
                    )
                items.append(lambda j=j, vt_tile=vt_tile: drain_v(j, vt_tile))
                return items

            # prologue: proj blocks 0+1 (dense PE, under the x DMA head);
            # thereafter iteration j prepares block j+2, so every drain DMA
            # reaches the sync FIFO with more than an iteration of slack
            for _pj in (0, 1):
                for it in proj_items(_pj):
                    it()
            for fn in deferred:
                fn()
            deferred.clear()
            load_x(2)
            load_x(3)

            def finalize_early(j, outp):
                """Release the outT PSUM bank and queue the XBAR."""
                oT = outT_sbs[j % 2]
                nc.vector.tensor_copy(oT[0:65, :], outp[:])
                ft = finp.tile([128, 4, 80], BF16, tag="ft")
                nc.sync.dma_start_transpose(ft[:], oT[:])
                return ft

            def finalize_late(j, ft):
                """Normalize by the transposed row sums and store."""
                rec = finp.tile([128, 4], F32, tag="rec")
                nc.vector.reciprocal(rec[:], ft[:, :, HS])
                o_f = finp.tile([128, 4, HS], F32, tag="o_f")
                for rr in range(4):
                    nc.vector.tensor_scalar_mul(
                        o_f[:, rr, :], ft[:, rr, 0:HS], rec[:, rr : rr + 1]
                    )
                nc.sync.dma_start(
                    out[j * TB : (j + 1) * TB, :].rearrange(
                        "(r p) h -> p r h", p=128
                    ),
                    o_f[:],
                )

            prev_out = None  # (j, psum tile) awaiting finalize
            for j in range(NJ):
                n_pairs = 2 * j + 2
                outp = ps_out.tile([HS + 1, TB], F32, tag="outT")

                # filler: projections for block j+2, then the previous
                # block's normalize+store, then the j+4 x-load
                filler = []
                if j + 2 < NJ:
                    filler = proj_items(j + 2)
                if prev_out is not None:
                    fj, fout = prev_out
                    ft = finalize_early(fj, fout)
                    filler.append(lambda fj=fj, ft=ft: finalize_late(fj, ft))
                if j + 4 < NJ:
                    filler.append(lambda jj=j + 4: load_x(jj))

                n_slots = n_pairs + LAG
                fill_slots = n_slots
                fi = 0  # filler cursor

                pair_state = {}
                for slot in range(n_slots):
                    if slot < n_pairs:
                        p = slot
                        iA, iB = 2 * p, 2 * p + 1
                        rA, rB = iA - 4 * j, iB - 4 * j
                        # A -> end of bank 0, B -> start of bank 1: the two
                        # row-group matmuls run concurrently in separate
                        # banks, and the valid region [TB-nA : TB+nB] stays
                        # contiguous for a single exp instruction
                        c0A = 128 * rA if rA > 0 else 0
                        c0B = 128 * rB if rB > 0 else 0
                        nA, nB = TB - c0A, TB - c0B
                        wei = ps_wei.tile([128, 2 * TB], F32, tag="wei")
                        nc.tensor.matmul(
                            wei[:, TB - nA : TB],
                            kTI[0:64, 128 * p : 128 * (p + 1)],
                            qkts[j][0:HS, c0A:TB],
                            start=True, stop=True,
                        )
                        nc.tensor.matmul(
                            wei[:, TB : TB + nB],
                            kTI[64:128, 128 * p : 128 * (p + 1)],
                            q2s[j][64:128, c0B:TB],
                            start=True, stop=True,
                        )
                        ex = expp.tile([128, 2 * TB], BF16, tag="exp")
                        nc.scalar.activation(
                            ex[:, TB - nA : TB + nB],
                            wei[:, TB - nA : TB + nB], EXP, scale=SCALE,
                        )
                        if rA >= 0:
                            nc.gpsimd.tensor_mul(
                                ex[:, TB - nA : TB - nA + 128],
                                ex[:, TB - nA : TB - nA + 128], mask_sb[:],
                            )
                        if rB >= 0:
                            nc.gpsimd.tensor_mul(
                                ex[:, TB : TB + 128], ex[:, TB : TB + 128],
                                mask_sb[:],
                            )
                        pair_state[p] = (ex, c0A, c0B, nA, nB, iA, iB)

                    if slot >= LAG:
                        p = slot - LAG
                        ex, c0A, c0B, nA, nB, iA, iB = pair_state.pop(p)
                        nc.tensor.matmul(
                            outp[:, c0A:TB], v_all[:, iA, :],
                            ex[:, TB - nA : TB],
                            start=(p == 0), stop=False,
                            skip_group_check=True,
                        )
                        nc.tensor.matmul(
                            outp[:, c0B:TB], v_all[:, iB, :],
                            ex[:, TB : TB + nB],
                            start=False, stop=(p == n_pairs - 1),
                            skip_group_check=True,
                        )

                    # spread proj filler across the early slots
                    want = min(len(filler), ((slot + 1) * len(filler)) // fill_slots)
                    while fi < want:
                        filler[fi]()
                        fi += 1

                while fi < len(filler):
                    filler[fi]()
                    fi += 1
                for fn in deferred:
                    fn()
                deferred.clear()
                prev_out = (j, outp)

            fj, fout = prev_out
            finalize_late(fj, finalize_early(fj, fout))

    nc.compile()
    return nc


_CACHE = {}


def _get_program():
    if "nc" not in _CACHE:
        _CACHE["nc"] = build_program()
    return _CACHE["nc"]


def _make_in_maps(inputs):
    import ml_dtypes

    x = np.asarray(inputs["x"], dtype=np.float32)
    Wq = np.asarray(inputs["Wq"], dtype=np.float32)
    Wk = np.asarray(inputs["Wk"], dtype=np.float32)
    Wv = np.asarray(inputs["Wv"], dtype=np.float32)
    # weights pre-arranged host-side to the SBUF layout [p, k, m] so the
    # const DMAs are contiguous full-rate transfers
    wqk = np.concatenate([Wq, Wk], axis=1).astype(ml_dtypes.bfloat16)
    wqk = np.ascontiguousarray(
        wqk.reshape(NK, 128, 128).transpose(1, 0, 2).reshape(128, NK * 128)
    )
    wv = Wv.astype(ml_dtypes.bfloat16)
    wv = np.ascontiguousarray(
        wv.reshape(NK, 128, HS).transpose(1, 0, 2).reshape(128, NK * HS)
    )
    mask = np.triu(np.ones((128, 128))).astype(ml_dtypes.bfloat16)
    in_maps = []
    for b in range(B):
        # xT block-major: [j][p][k][t] so each block load is one contiguous
        # 8KB-per-partition transfer
        xb = x[b].T.astype(ml_dtypes.bfloat16)  # [C, T]
        xp = xb.reshape(NK, 128, NJ, TB).transpose(2, 1, 0, 3)
        xp = np.ascontiguousarray(xp.reshape(NJ, 128, NK * TB))
        in_maps.append({"xTp": xp, "wqk": wqk, "wv": wv, "mask": mask})
    return in_maps


def kernel(x, Wk, Wq, Wv):
    nc = _get_program()
    in_maps = _make_in_maps({"x": x, "Wq": Wq, "Wk": Wk, "Wv": Wv})
    res = bass_utils.run_bass_kernel_spmd(nc, in_maps, core_ids=list(range(B)))
    return np.stack([res.results[b]["out"] for b in range(B)], axis=0)
